# revision 2
# baseline (speedup 1.0000x reference)
"""Multi-head attention (B=2, S=2048, E=1024, H=16) on 8 TRN2 NeuronCores.

Sharding: batch x head-group. Core c handles batch b=c//4 and head group
g=c%4 (4 heads = 256 of E). Each core computes its heads' attention output
slice and a partial fc_out product [S, E]; the host sums the 4 partials per
batch and adds b_out.

Device-side math per core (all matmuls in float32r, full PE rate):
  qpT = (Wq_g @ q[b].T + bq)      [256, S]   (T layout: dims on partitions)
  kpT = (Wk_g @ k_c[b].T + bk)    [256, SKV] (k compressed by mask, padded)
  vp  = (v_c[b] @ Wv_g.T + bv)*m  [SKV, 4*65] (per head: 64 dims + ones col)
  S_T = kpT_h.T-chunks @ qpT_h    [SKV, S] per head (2 heads row-packed, K=64)
  E_T = exp(S_T)                  (no max-subtraction: |energy| <~ 60, safe)
  AV  = vp_aug.T @ E_T  -> [65, S]: rows 0-63 = unnormalized O_T, row 64 =
        softmax denominator (ones-column trick; pad rows contribute 0)
  O_T = AV[0:64] / AV[64]         (recip + gpsimd partition-broadcast)
  out_partial = O_T.T @ Wo_g.T    [S, E]

Mask handling is exact: masked K/V rows are removed on the host (gather),
so softmax(where(mask==0, -1e20, e)) == exp(e_valid)/sum(exp(e_valid)).
"""

import os

import numpy as np

B, S, E, H = 2, 2048, 1024, 16
D = E // H           # 64
NCORES = 8
GROUPS = 4           # head groups per batch (cores per batch)
HPG = H // GROUPS    # 4 heads per core
DC = E // GROUPS     # 256 dims per core
NB = E // 128        # 8 contraction chunks over E
SQB = 256            # sq block width for scores/AV
NSQB = S // SQB      # 8

_CACHE = {}


def _split_excess_waits(nc, max_waits=1):
    """walrus rejects instructions carrying >1 sem wait; spread extras onto
    single-wait NoOps inserted before the instruction on the same engine."""
    import concourse.mybir as mybir

    n_split = 0
    for f in nc.m.functions:
        for bb in f.blocks:
            out, changed = [], False
            for ins in bb.instructions:
                si = ins.sync_info
                if si is not None and si.on_wait is not None and len(si.on_wait) > max_waits:
                    waits = list(si.on_wait)
                    for w in waits[:-max_waits]:
                        out.append(mybir.InstNoOp(
                            name=nc.get_next_instruction_name(),
                            engine=ins.engine, ins=[], outs=[],
                            sync_info=mybir.SyncInfo(on_wait=[w], on_update=[])))
                        n_split += 1
                    ins.sync_info = mybir.SyncInfo(
                        on_wait=waits[-max_waits:], on_update=list(si.on_update))
                    changed = True
                out.append(ins)
            if changed:
                bb.instructions = out
    return n_split


def _build(skv, split_waits=True):
    import concourse.bass as bass
    import concourse.mybir as mybir
    import concourse.tile as tile

    f32 = mybir.dt.float32
    f32r = mybir.dt.float32r
    f16 = mybir.dt.float16
    bf16 = mybir.dt.bfloat16
    Alu = mybir.AluOpType
    Act = mybir.ActivationFunctionType

    nsk = skv // 128
    kblocks = []
    rem = skv
    while rem > 0:
        w = 384 if rem % 384 == 0 else min(256, rem)
        kblocks.append(w)
        rem -= w

    nc = bass.Bass()
    xqT = nc.declare_dram_parameter("xqT", [E, S], f32r, isOutput=False)
    xkT = nc.declare_dram_parameter("xkT", [E, skv], f32r, isOutput=False)
    xvT = nc.declare_dram_parameter("xvT", [E, skv], f16, isOutput=False)
    wqT = nc.declare_dram_parameter("wqT", [E, DC], f32r, isOutput=False)
    wkT = nc.declare_dram_parameter("wkT", [E, DC], f32r, isOutput=False)
    wvT = nc.declare_dram_parameter("wvT", [E, DC], f16, isOutput=False)
    woT = nc.declare_dram_parameter("woT", [DC, E], f16, isOutput=False)
    bq_d = nc.declare_dram_parameter("bq", [DC], f32, isOutput=False)
    bk_d = nc.declare_dram_parameter("bk", [DC], f32, isOutput=False)
    bv_d = nc.declare_dram_parameter("bv", [DC], f32, isOutput=False)
    vm_d = nc.declare_dram_parameter("vmask", [skv], f32, isOutput=False)
    ones_d = nc.declare_dram_parameter("ones64", [1, 64], f32r, isOutput=False)
    out_d = nc.declare_dram_parameter("out", [2, S, E], f16, isOutput=True)
    srow_d = nc.dram_tensor("srow", [2, 2, S], f32)
    rrow_d = nc.dram_tensor("rrow", [2, 2, S], f32r)

    xqT_r = xqT.rearrange("(ko p) s -> p ko s", p=128)
    xkT_r = xkT.rearrange("(ko p) s -> p ko s", p=128)
    xvT_r = xvT.rearrange("(ko p) s -> p ko s", p=128)

    QB = 512

    with tile.TileContext(nc) as tc:
        with (
            tc.tile_pool(name="weights", bufs=4) as wpool,
            tc.tile_pool(name="consts", bufs=1) as cpool,
            tc.tile_pool(name="persist", bufs=1) as ppool,
            tc.tile_pool(name="small", bufs=2) as smpool,
            tc.tile_pool(name="proj_ps", bufs=1, space="PSUM") as pps,
            tc.tile_pool(name="stream", bufs=2) as spool,
            tc.tile_pool(name="att_ps", bufs=2, space="PSUM") as aps,
            tc.tile_pool(name="av_ps", bufs=1, space="PSUM") as avps,
            tc.tile_pool(name="fc_ps", bufs=2, space="PSUM") as fps,
            tc.tile_pool(name="et", bufs=3) as etpool,
            tc.tile_pool(name="outp", bufs=3) as opool,
            tc.tile_pool(name="sums", bufs=2) as supool,
            tc.tile_pool(name="rcr", bufs=1) as rcpool,
        ):
            # ---- weights / constants (k first: kpT gates attention) ----
            wk_t = wpool.tile([128, NB, DC], f32r, tag="w", name="wk_t")
            wq_t = wpool.tile([128, NB, DC], f32r, tag="w", name="wq_t")
            wv_t = wpool.tile([128, NB, DC], f16, tag="w", name="wv_t")
            wo_t = wpool.tile([128, DC // 128, E], f16, tag="w", name="wo_t")
            nc.sync.dma_start(wk_t[:], wkT.rearrange("(ko p) m -> p ko m", p=128))
            bq_t = cpool.tile([128, 2], f32, tag="bq")
            bk_t = cpool.tile([128, 2], f32, tag="bk")
            bv_t = cpool.tile([128, DC], f32, tag="bv")
            vm_t = cpool.tile([128, nsk], f32, tag="vm")
            nc.sync.dma_start(bk_t[:], bk_d.rearrange("(c p) -> p c", p=128))
            nc.sync.dma_start(bq_t[:], bq_d.rearrange("(c p) -> p c", p=128))
            nc.sync.dma_start(bv_t[:], bv_d[None, :].to_broadcast((128, DC)))
            nc.sync.dma_start(vm_t[:], vm_d.rearrange("(s p) -> p s", p=128))
            ones_t = cpool.tile([1, 64], f32r, tag="ones")
            nc.sync.dma_start(ones_t[:], ones_d[:])

            qpT = ppool.tile([128, 2, S], f32r, tag="qpT")
            kpT = ppool.tile([128, 2, skv], f32r, tag="kpT")
            vp = ppool.tile([128, nsk, HPG * (D + 1)], bf16, tag="vp")
            o_un = ppool.tile([128, 2, S], f32, tag="o_un")
            o_f16 = ppool.tile([128, 2, S], f16, tag="o_f16")

            def proj_k():
                off = 0
                for w in kblocks:
                    xk = spool.tile([128, NB, max(kblocks)], f32r, tag="xk", name="xk")
                    nc.sync.dma_start(xk[:, :, :w], xkT_r[:, :, off:off + w])
                    for mc in range(2):
                        ps = pps.tile([128, 512], f32, tag="pp", name="kp_ps")[:, :max(kblocks)]
                        for kc in range(NB):
                            nc.tensor.matmul(
                                ps[:, :w], wk_t[:, kc, mc * 128:(mc + 1) * 128],
                                xk[:, kc, :w], start=(kc == 0), stop=(kc == NB - 1))
                        nc.vector.tensor_tensor(
                            out=kpT[:, mc, off:off + w], in0=ps[:, :w],
                            in1=bk_t[:, mc:mc + 1].to_broadcast((128, w)), op=Alu.add)
                    off += w

            def proj_q(nb):
                xq = spool.tile([128, NB, 512], f32r, tag="xq", name="xq")
                nc.sync.dma_start(xq[:], xqT_r[:, :, nb * 512:(nb + 1) * 512])
                for mc in range(2):
                    ps = pps.tile([128, 512], f32, tag="pp", name="qp_ps")
                    for kc in range(NB):
                        nc.tensor.matmul(
                            ps[:], wq_t[:, kc, mc * 128:(mc + 1) * 128],
                            xq[:, kc, :], start=(kc == 0), stop=(kc == NB - 1))
                    nc.vector.tensor_tensor(
                        out=qpT[:, mc, nb * 512:(nb + 1) * 512], in0=ps[:],
                        in1=bq_t[:, mc:mc + 1].to_broadcast((128, 512)), op=Alu.add)

            def proj_v(sc):
                xv = spool.tile([128, NB, 128], f16, tag="xv", name="xv")
                nc.sync.dma_start(xv[:], xvT_r[:, :, sc * 128:(sc + 1) * 128])
                ps = pps.tile([128, 512], f32, tag="pp", name="vp_ps")[:, :DC]
                for kc in range(NB):
                    nc.tensor.matmul(
                        ps[:], xv[:, kc, :], wv_t[:, kc, :],
                        start=(kc == 0), stop=(kc == NB - 1))
                t1 = smpool.tile([128, DC], f32, tag="vtmp")
                nc.vector.tensor_tensor(out=t1[:], in0=ps[:], in1=bv_t[:], op=Alu.add)
                vps = vp[:, sc, :].rearrange("p (h w) -> p h w", w=D + 1)
                nc.vector.tensor_tensor(
                    out=vps[:, :, 0:D],
                    in0=t1.rearrange("p (h w) -> p h w", w=D),
                    in1=vm_t[:, sc:sc + 1, None].to_broadcast((128, HPG, D)),
                    op=Alu.mult)
                nc.vector.tensor_copy(
                    out=vps[:, :, D:D + 1],
                    in_=vm_t[:, sc:sc + 1, None].to_broadcast((128, HPG, 1)))

            # lead-in: kpT fully, first qpT block, vp
            proj_k()
            nc.sync.dma_start(wq_t[:], wqT.rearrange("(ko p) m -> p ko m", p=128))
            proj_q(0)
            nc.sync.dma_start(wv_t[:], wvT.rearrange("(ko p) m -> p ko m", p=128))
            for sc in range(nsk):
                proj_v(sc)
            nc.sync.dma_start(wo_t[:], woT.rearrange("(ko p) n -> p ko n", p=128))

            for pt in range(2):
                sums = [supool.tile([1, S], f32, tag="sums", name=f"sums{j}")
                        for j in range(2)]
                for qb in range(S // QB):
                    q0 = qb * QB
                    et = [etpool.tile([128, nsk, QB], bf16, tag="et", name=f"et{j}")
                          for j in range(2)]
                    for skc in range(nsk):
                        psx = [aps.tile([128, QB], f32, tag=f"sc{j}", name=f"psx{j}")
                               for j in range(2)]
                        for j in range(2):
                            nc.tensor.matmul(
                                psx[j][:],
                                kpT[64 * j:64 * j + 64, pt, skc * 128:(skc + 1) * 128],
                                qpT[64 * j:64 * j + 64, pt, q0:q0 + QB],
                                start=True, stop=True, tile_position=(64 * j, 0))
                        for j in range(2):
                            nc.scalar.activation(et[j][:, skc, :], psx[j][:], Act.Exp)
                    for j in range(2):
                        hl = 2 * pt + j
                        ps_av = avps.tile([D + 1, QB], f32, tag="av")
                        for skc in range(nsk):
                            nc.tensor.matmul(
                                ps_av[:], vp[:, skc, hl * (D + 1):(hl + 1) * (D + 1)],
                                et[j][:, skc, :],
                                start=(skc == 0), stop=(skc == nsk - 1))
                        nc.vector.tensor_copy(
                            out=o_un[64 * j:64 * j + 64, pt, q0:q0 + QB],
                            in_=ps_av[0:D, :])
                        nc.vector.tensor_copy(
                            out=sums[j][0:1, q0:q0 + QB], in_=ps_av[D:D + 1, :])
                    # trailing qpT projection blocks interleave with attention
                    # pt=0: full-array matmuls keep the PE dense (and warm).
                    if pt == 0 and qb + 1 < S // QB:
                        proj_q(qb + 1)
                # normalize pair pt
                for j in range(2):
                    s128 = smpool.tile([128, S // 128], f32, tag="s128")
                    nc.sync.dma_start(s128[:], sums[j][0:1, :])
                    nc.vector.reciprocal(out=s128[:], in_=s128[:])
                    r128 = smpool.tile([128, S // 128], f32r, tag="r128")
                    nc.vector.tensor_copy(out=r128[:], in_=s128[:])
                    rc_r = rcpool.tile([1, S], f32r, tag="rcr")
                    nc.sync.dma_start(rc_r[0:1, :], r128[:])
                    for qb in range(S // 512):
                        rc_ps = avps.tile([64, 512], f32, tag="av", name="rc_ps")
                        nc.tensor.matmul(
                            rc_ps[:], ones_t[:], rc_r[0:1, qb * 512:(qb + 1) * 512],
                            start=True, stop=True)
                        nc.vector.tensor_tensor(
                            out=o_f16[64 * j:64 * j + 64, pt, qb * 512:(qb + 1) * 512],
                            in0=o_un[64 * j:64 * j + 64, pt, qb * 512:(qb + 1) * 512],
                            in1=rc_ps[:], op=Alu.mult)
                # fc_out pass for this pair
                for sqc in range(S // 128):
                    for eb in range(2):
                        ps = fps.tile([128, 512], f32, tag="fc")
                        nc.tensor.matmul(
                            ps[:], o_f16[:, pt, sqc * 128:(sqc + 1) * 128],
                            wo_t[:, pt, eb * 512:(eb + 1) * 512],
                            start=True, stop=True)
                        ob = opool.tile([128, 512], f16, tag="ob")
                        nc.any.tensor_copy(out=ob[:], in_=ps[:])
                        nc.sync.dma_start(
                            out_d[pt, sqc * 128:(sqc + 1) * 128,
                                  eb * 512:(eb + 1) * 512],
                            ob[:])

    if split_waits:
        _split_excess_waits(nc)
    return nc


def _prep_inputs(q, k, v, mask, W_qkv, b_qkv, W_out, b_out):
    """Host-side shard/layout prep. Returns (skv, in_maps)."""
    q = np.asarray(q, dtype=np.float32)
    k = np.asarray(k, dtype=np.float32)
    v = np.asarray(v, dtype=np.float32)
    mask = np.asarray(mask)
    W_qkv = np.asarray(W_qkv, dtype=np.float32)
    b_qkv = np.asarray(b_qkv, dtype=np.float32)
    W_out = np.asarray(W_out, dtype=np.float32)

    valid = [np.nonzero(mask[b, 0, 0] != 0)[0] for b in range(B)]
    cnts = [len(vi) for vi in valid]
    skv = max(128, max((c + 127) // 128 * 128 for c in cnts))

    # per-batch tensors
    qT, kTc, vTc, vms = [], [], [], []
    for b in range(B):
        qT.append(np.ascontiguousarray(q[b].T))
        kt = np.zeros((E, skv), np.float32)
        vt = np.zeros((E, skv), np.float16)
        kt[:, :cnts[b]] = k[b][valid[b]].T
        vt[:, :cnts[b]] = v[b][valid[b]].T
        kTc.append(kt)
        vTc.append(vt)
        vm = np.zeros((skv,), np.float32)
        vm[:cnts[b]] = 1.0
        vms.append(vm)

    in_maps = []
    for c in range(NCORES):
        b, g = divmod(c, GROUPS)
        sl = slice(g * DC, (g + 1) * DC)
        in_maps.append({
            "xqT": qT[b], "xkT": kTc[b], "xvT": vTc[b],
            "wqT": np.ascontiguousarray(W_qkv[sl, :].T),
            "wkT": np.ascontiguousarray(W_qkv[E:][sl, :].T),
            "wvT": np.ascontiguousarray(W_qkv[2 * E:][sl, :].T).astype(np.float16),
            "woT": np.ascontiguousarray(W_out[:, sl].T).astype(np.float16),
            "bq": np.ascontiguousarray(b_qkv[sl]),
            "bk": np.ascontiguousarray(b_qkv[E:][sl]),
            "bv": np.ascontiguousarray(b_qkv[2 * E:][sl]),
            "vmask": vms[b],
            "ones64": np.ones((1, 64), np.float32),
        })
    return skv, in_maps


def kernel(q, k, v, mask, W_qkv, b_qkv, W_out, b_out):
    from concourse import bass_utils

    skv, in_maps = _prep_inputs(q, k, v, mask, W_qkv, b_qkv, W_out, b_out)
    if skv not in _CACHE:
        _CACHE[skv] = _build(skv)
    nc = _CACHE[skv]

    trace = os.environ.get("KERNEL_TRACE") == "1"
    if trace:
        bass_utils.upload_artifacts = lambda tmpdir: "local://" + tmpdir
    res = bass_utils.run_bass_kernel_spmd(
        nc, in_maps, list(range(NCORES)), trace=trace)
    if trace:
        global LAST_RES
        LAST_RES = res
        print(f"HW exec time: {res.exec_time_ns} ns")

    b_out = np.asarray(b_out, dtype=np.float32)
    out = np.zeros((B, S, E), np.float32)
    for c in range(NCORES):
        out[c // GROUPS] += res.results[c]["out"].astype(np.float32).sum(axis=0)
    out += b_out[None, None, :]
    return out



# revision 7
# speedup vs baseline: 1.0542x; 1.0542x over previous
"""Multi-head attention (B=2, S=2048, E=1024, H=16) on 8 TRN2 NeuronCores.

Sharding: batch x head-group. Core c handles batch b=c//4 and head group
g=c%4 (4 heads = 256 of E). Each core computes its heads' attention output
slice and a partial fc_out product [S, E]; the host sums the 4 partials per
batch and adds b_out.

Device-side math per core:
  qpT = (Wq_g @ q[b].T + bq)      [256, S]   f32r (dims on partitions)
  kpT = (Wk_g @ k_c[b].T + bk)    [256, SKV] f32r (k compressed by mask)
  vp  = (v_c[b] @ Wv_g.T + bv)*m  [SKV, 4*65] bf16 (64 dims + ones col/head)
  per (qb, pt): S_T chunks [128kv, 512q] for both heads of the pair land in
  one 2-bank psum tile -> one Exp activation [128, 1024] -> et bf16; AV
  accumulates vp_aug.T @ E_T -> [65, 512] (row 64 = softmax denominator).
  normalize: Pool copies AV psum -> bf16; a [1,64] ones matmul broadcasts
  the denominator row; DVE reciprocal + multiply -> o_bf f16.
  fc_out accumulates BOTH pairs into one psum tile -> out [S, E] f16.

The loop is qb-major with fc(qb-1) and proj_q(qb+1) matmuls interleaved
into the attention stream so the PE never idles (p-state stays ramped).

Mask handling is exact: masked K/V rows are removed on the host (gather),
so softmax(where(mask==0, -1e20, e)) == exp(e_valid)/sum(exp(e_valid)).
"""

import os

import numpy as np

B, S, E, H = 2, 2048, 1024, 16
D = E // H           # 64
NCORES = 8
GROUPS = 4           # head groups per batch (cores per batch)
HPG = H // GROUPS    # 4 heads per core
DC = E // GROUPS     # 256 dims per core
NB = E // 128        # 8 contraction chunks over E
QB = 512             # query block
NQB = S // QB        # 4

_CACHE = {}


def _split_excess_waits(nc, max_waits=1):
    """walrus rejects instructions carrying >1 sem wait; spread extras onto
    single-wait NoOps inserted before the instruction on the same engine."""
    import concourse.mybir as mybir

    n_split = 0
    for f in nc.m.functions:
        for bb in f.blocks:
            out, changed = [], False
            for ins in bb.instructions:
                si = ins.sync_info
                if si is not None and si.on_wait is not None and len(si.on_wait) > max_waits:
                    waits = list(si.on_wait)
                    for w in waits[:-max_waits]:
                        out.append(mybir.InstNoOp(
                            name=nc.get_next_instruction_name(),
                            engine=ins.engine, ins=[], outs=[],
                            sync_info=mybir.SyncInfo(on_wait=[w], on_update=[])))
                        n_split += 1
                    ins.sync_info = mybir.SyncInfo(
                        on_wait=waits[-max_waits:], on_update=list(si.on_update))
                    changed = True
                out.append(ins)
            if changed:
                bb.instructions = out
    return n_split


def _build(skv, split_waits=True):
    import concourse.bass as bass
    import concourse.mybir as mybir
    import concourse.tile as tile

    f32 = mybir.dt.float32
    f32r = mybir.dt.float32r
    f16 = mybir.dt.float16
    bf16 = mybir.dt.bfloat16
    Alu = mybir.AluOpType
    Act = mybir.ActivationFunctionType

    nsk = skv // 128

    nc = bass.Bass()
    xqT = nc.declare_dram_parameter("xqT", [E, S], f16, isOutput=False)
    xkT = nc.declare_dram_parameter("xkT", [E, skv], f16, isOutput=False)
    xvT = nc.declare_dram_parameter("xvT", [E, skv], f16, isOutput=False)
    wqT = nc.declare_dram_parameter("wqT", [E, DC], f16, isOutput=False)
    wkT = nc.declare_dram_parameter("wkT", [E, DC], f16, isOutput=False)
    wvT = nc.declare_dram_parameter("wvT", [E, DC], f16, isOutput=False)
    woT = nc.declare_dram_parameter("woT", [DC, E], f16, isOutput=False)
    bq_d = nc.declare_dram_parameter("bq", [DC], f32, isOutput=False)
    bk_d = nc.declare_dram_parameter("bk", [DC], f32, isOutput=False)
    bv_d = nc.declare_dram_parameter("bv", [DC], f32, isOutput=False)
    vm_d = nc.declare_dram_parameter("vmask", [skv], f32, isOutput=False)
    out_d = nc.declare_dram_parameter("out", [S, E], f16, isOutput=True)

    xqT_r = xqT.rearrange("(ko p) s -> p ko s", p=128)
    xkT_r = xkT.rearrange("(ko p) s -> p ko s", p=128)
    xvT_r = xvT.rearrange("(ko p) s -> p ko s", p=128)

    with tile.TileContext(nc) as tc:
        with (
            tc.tile_pool(name="weights", bufs=4) as wpool,
            tc.tile_pool(name="consts", bufs=1) as cpool,
            tc.tile_pool(name="persist", bufs=1) as ppool,
            tc.tile_pool(name="xq", bufs=2) as xqpool,
            tc.tile_pool(name="xkv", bufs=1) as xkvpool,
            tc.tile_pool(name="sc_ps", bufs=2, space="PSUM") as scps,
            tc.tile_pool(name="av_ps", bufs=2, space="PSUM") as avps,
            tc.tile_pool(name="fc_ps", bufs=2, space="PSUM") as fps,
            tc.tile_pool(name="et", bufs=3) as etpool,
            tc.tile_pool(name="ou", bufs=3) as oupool,
            tc.tile_pool(name="rec", bufs=3) as rcpool,
            tc.tile_pool(name="outp", bufs=2) as opool,
            tc.tile_pool(name="small", bufs=2) as smpool,
        ):
            # ---- constants / weights (k first: kpT gates attention) ----
            wk_t = wpool.tile([128, NB, DC], f16, tag="w", name="wk_t")
            wq_t = wpool.tile([128, NB, DC], f16, tag="w", name="wq_t")
            wv_t = wpool.tile([128, NB, DC], f16, tag="w", name="wv_t")
            wo_t = wpool.tile([128, DC // 128, E], f16, tag="w", name="wo_t")
            nc.sync.dma_start(wk_t[:], wkT.rearrange("(ko p) m -> p ko m", p=128))

            xk = xkvpool.tile([128, NB, skv], f16, tag="xk", name="xk")
            KCH = 384 if skv % 384 == 0 else 128
            nkch = skv // KCH
            for kb in range(nkch):
                nc.sync.dma_start(xk[:, :, kb * KCH:(kb + 1) * KCH],
                                  xkT_r[:, :, kb * KCH:(kb + 1) * KCH])

            nc.sync.dma_start(wq_t[:], wqT.rearrange("(ko p) m -> p ko m", p=128))
            xq = [xqpool.tile([128, NB, QB], f16, tag="xq", name=f"xq{nb}")
                  for nb in range(NQB)]
            nc.sync.dma_start(xq[0][:], xqT_r[:, :, 0:QB])
            nc.sync.dma_start(wv_t[:], wvT.rearrange("(ko p) m -> p ko m", p=128))
            xv = xkvpool.tile([128, NB, skv], f16, tag="xv", name="xv")
            nc.sync.dma_start(xv[:], xvT_r[:])
            nc.sync.dma_start(xq[1][:], xqT_r[:, :, QB:2 * QB])

            bq_t = cpool.tile([128, 2], f32, tag="bq")
            bk_t = cpool.tile([128, 2], f32, tag="bk")
            bv_t = cpool.tile([128, DC], f32, tag="bv")
            vm_t = cpool.tile([128, nsk], f32, tag="vm")
            nc.sync.dma_start(bk_t[:], bk_d.rearrange("(c p) -> p c", p=128))
            nc.sync.dma_start(bq_t[:], bq_d.rearrange("(c p) -> p c", p=128))
            nc.sync.dma_start(bv_t[:], bv_d[None, :].to_broadcast((128, DC)))
            nc.sync.dma_start(vm_t[:], vm_d.rearrange("(s p) -> p s", p=128))
            nc.sync.dma_start(xq[2][:], xqT_r[:, :, 2 * QB:3 * QB])
            nc.sync.dma_start(xq[3][:], xqT_r[:, :, 3 * QB:4 * QB])
            nc.sync.dma_start(wo_t[:], woT.rearrange("(ko p) n -> p ko n", p=128))

            ones_t = cpool.tile([128, 64], bf16, tag="ones")
            nc.gpsimd.memset(ones_t[:], 1.0)

            qpT = ppool.tile([128, 2, S], f32r, tag="qpT")
            kpT = ppool.tile([128, 2, skv], f32r, tag="kpT")
            vp = ppool.tile([128, nsk, HPG, D + 1], bf16, tag="vp")
            o_bf = ppool.tile([128, 2, S], f16, tag="o_bf")

            # ---- projection emitters ----
            def proj_k(kb):
                off = kb * KCH
                for mc in range(2):
                    ps = fps.tile([128, 512], f32, tag="fc", name="kp_ps")[:, :KCH]
                    for kc in range(NB):
                        nc.tensor.matmul(
                            ps[:], wk_t[:, kc, mc * 128:(mc + 1) * 128],
                            xk[:, kc, off:off + KCH],
                            start=(kc == 0), stop=(kc == NB - 1))
                    nc.vector.tensor_tensor(
                        out=kpT[:, mc, off:off + KCH], in0=ps[:],
                        in1=bk_t[:, mc:mc + 1].to_broadcast((128, KCH)), op=Alu.add)

            def proj_q_mms(nb, mc, ps):
                """8 accumulating matmuls for qpT block nb, half mc."""
                for kc in range(NB):
                    yield lambda kc=kc: nc.tensor.matmul(
                        ps[:], wq_t[:, kc, mc * 128:(mc + 1) * 128],
                        xq[nb][:, kc, :], start=(kc == 0), stop=(kc == NB - 1))

            def proj_q_bias(nb, mc, ps):
                nc.vector.tensor_tensor(
                    out=qpT[:, mc, nb * QB:(nb + 1) * QB], in0=ps[:],
                    in1=bq_t[:, mc:mc + 1].to_broadcast((128, QB)), op=Alu.add)

            def proj_q(nb):
                for mc in range(2):
                    ps = fps.tile([128, 512], f32, tag="fc", name="qp_ps")
                    for mm in proj_q_mms(nb, mc, ps):
                        mm()
                    proj_q_bias(nb, mc, ps)

            def proj_v(sc):
                ps = fps.tile([128, 512], f32, tag="fc", name="vp_ps")[:, :DC]
                for kc in range(NB):
                    nc.tensor.matmul(
                        ps[:], xv[:, kc, sc * 128:(sc + 1) * 128], wv_t[:, kc, :],
                        start=(kc == 0), stop=(kc == NB - 1))
                t1 = smpool.tile([128, DC], f32, tag="vtmp")
                nc.vector.tensor_tensor(out=t1[:], in0=ps[:], in1=bv_t[:], op=Alu.add)
                nc.gpsimd.tensor_tensor(
                    out=vp[:, sc, :, 0:D],
                    in0=t1.rearrange("p (h w) -> p h w", w=D),
                    in1=vm_t[:, sc:sc + 1, None].to_broadcast((128, HPG, D)),
                    op=Alu.mult)
                nc.gpsimd.tensor_copy(
                    out=vp[:, sc, :, D:D + 1],
                    in_=vm_t[:, sc:sc + 1, None].to_broadcast((128, HPG, 1)))

            # ---- lead-in: kpT, qpT block 0, vp ----
            for kb in range(nkch):
                proj_k(kb)
            proj_q(0)
            for sc in range(nsk):
                proj_v(sc)

            # ---- fc_out emitter (query row sqc: 128 queries, eb: 512 cols) --
            def fc_mms(sqc, eb, ps):
                for pt in range(2):
                    yield lambda pt=pt: nc.tensor.matmul(
                        ps[:], o_bf[:, pt, sqc * 128:(sqc + 1) * 128],
                        wo_t[:, pt, eb * 512:(eb + 1) * 512],
                        start=(pt == 0), stop=(pt == 1))

            def fc_tail(sqc, eb, ps, ob):
                nc.vector.tensor_copy(out=ob[:, eb * 512:(eb + 1) * 512], in_=ps[:])
                if eb == 1:
                    nc.sync.dma_start(
                        out_d[sqc * 128:(sqc + 1) * 128, :], ob[:])

            def fc_block(qb):
                """All fc work for query block qb as a list of thunks
                (each one PE matmul or a tail), to interleave."""
                thunks = []
                for sq in range(QB // 128):
                    sqc = qb * (QB // 128) + sq
                    ob = opool.tile([128, E], f16, tag="ob", name="ob")
                    for eb in range(2):
                        ps = fps.tile([128, 512], f32, tag="fc", name="fc_ps")
                        for mm in fc_mms(sqc, eb, ps):
                            thunks.append(mm)
                        thunks.append(
                            lambda sqc=sqc, eb=eb, ps=ps, ob=ob: fc_tail(sqc, eb, ps, ob))
                return thunks

            # ---- attention ----
            for qb in range(NQB):
                q0 = qb * QB
                # interleavable PE work: fc of previous block + proj of next q
                extra = fc_block(qb - 1) if qb > 0 else []
                if qb + 1 < NQB:
                    for mc in range(2):
                        ps = fps.tile([128, 512], f32, tag="fc", name="qp_ps")
                        for mm in proj_q_mms(qb + 1, mc, ps):
                            extra.append(mm)
                        extra.append(
                            lambda nb=qb + 1, mc=mc, ps=ps: proj_q_bias(nb, mc, ps))
                ei = 0

                def drain(n):
                    nonlocal ei
                    for _ in range(n):
                        if ei < len(extra):
                            extra[ei]()
                            ei += 1

                for pt in range(2):
                    ps_av = [avps.tile([D + 1, QB], f32, tag="av", name=f"av{j}")
                             for j in range(2)]
                    ets = []
                    for skc in range(nsk):
                        psx = scps.tile([128, 2, QB], f32, tag="sc", name="psx")
                        for j in range(2):
                            nc.tensor.matmul(
                                psx[:, j, :],
                                kpT[64 * j:64 * j + 64, pt, skc * 128:(skc + 1) * 128],
                                qpT[64 * j:64 * j + 64, pt, q0:q0 + QB],
                                start=True, stop=True, tile_position=(64 * j, 0))
                        et = etpool.tile([128, 2, QB], bf16, tag="et", name="et")
                        nc.scalar.activation(et[:], psx[:], Act.Exp)
                        ets.append(et)
                        # AV for previous chunk (keeps one step of slack
                        # between PE and the exp on the scalar engine)
                        if skc > 0:
                            for j in range(2):
                                nc.tensor.matmul(
                                    ps_av[j][:], vp[:, skc - 1, 2 * pt + j, :],
                                    ets[skc - 1][:, j, :],
                                    start=(skc - 1 == 0), stop=False)
                        drain(2)
                    for j in range(2):
                        nc.tensor.matmul(
                            ps_av[j][:], vp[:, nsk - 1, 2 * pt + j, :],
                            ets[nsk - 1][:, j, :],
                            start=False, stop=True)
                    # normalize: psum -> bf16, broadcast denom, recip, mult
                    for j in range(2):
                        ou = oupool.tile([D + 1, QB], bf16, tag="ou", name="ou")
                        nc.vector.tensor_copy(out=ou[:], in_=ps_av[j][:])
                        bc = avps.tile([D + 1, QB], f32, tag="av", name="bc")
                        nc.tensor.matmul(
                            bc[0:64, :], ones_t[64:65, :], ou[64:65, :],
                            start=True, stop=True)
                        rec = rcpool.tile([64, QB], bf16, tag="rec", name="rec")
                        with nc.allow_low_precision(reason="softmax denom recip; bf16 ok"):
                            nc.vector.reciprocal(out=rec[:], in_=bc[0:64, :])
                        nc.gpsimd.tensor_tensor(
                            out=o_bf[64 * j:64 * j + 64, pt, q0:q0 + QB],
                            in0=ou[0:D, :], in1=rec[:], op=Alu.mult)
                    drain(4)
                # any leftover interleaved work
                drain(len(extra))
            # final fc block
            for th in fc_block(NQB - 1):
                th()

    if split_waits:
        _split_excess_waits(nc)
    return nc


def _prep_inputs(q, k, v, mask, W_qkv, b_qkv, W_out, b_out):
    """Host-side shard/layout prep. Returns (skv, in_maps)."""
    q = np.asarray(q, dtype=np.float32)
    k = np.asarray(k, dtype=np.float32)
    v = np.asarray(v, dtype=np.float32)
    mask = np.asarray(mask)
    W_qkv = np.asarray(W_qkv, dtype=np.float32)
    b_qkv = np.asarray(b_qkv, dtype=np.float32)
    W_out = np.asarray(W_out, dtype=np.float32)

    valid = [np.nonzero(mask[b, 0, 0] != 0)[0] for b in range(B)]
    cnts = [len(vi) for vi in valid]
    skv = max(128, max((c + 127) // 128 * 128 for c in cnts))

    qT, kTc, vTc, vms = [], [], [], []
    for b in range(B):
        qT.append(np.ascontiguousarray(q[b].T).astype(np.float16))
        kt = np.zeros((E, skv), np.float16)
        vt = np.zeros((E, skv), np.float16)
        kt[:, :cnts[b]] = k[b][valid[b]].T
        vt[:, :cnts[b]] = v[b][valid[b]].T
        kTc.append(kt)
        vTc.append(vt)
        vm = np.zeros((skv,), np.float32)
        vm[:cnts[b]] = 1.0
        vms.append(vm)

    in_maps = []
    for c in range(NCORES):
        b, g = divmod(c, GROUPS)
        sl = slice(g * DC, (g + 1) * DC)
        in_maps.append({
            "xqT": qT[b], "xkT": kTc[b], "xvT": vTc[b],
            "wqT": np.ascontiguousarray(W_qkv[sl, :].T).astype(np.float16),
            "wkT": np.ascontiguousarray(W_qkv[E:][sl, :].T).astype(np.float16),
            "wvT": np.ascontiguousarray(W_qkv[2 * E:][sl, :].T).astype(np.float16),
            "woT": np.ascontiguousarray(W_out[:, sl].T).astype(np.float16),
            "bq": np.ascontiguousarray(b_qkv[sl]),
            "bk": np.ascontiguousarray(b_qkv[E:][sl]),
            "bv": np.ascontiguousarray(b_qkv[2 * E:][sl]),
            "vmask": vms[b],
        })
    return skv, in_maps


def kernel(q, k, v, mask, W_qkv, b_qkv, W_out, b_out):
    from concourse import bass_utils

    skv, in_maps = _prep_inputs(q, k, v, mask, W_qkv, b_qkv, W_out, b_out)
    if skv not in _CACHE:
        _CACHE[skv] = _build(skv)
    nc = _CACHE[skv]

    trace = os.environ.get("KERNEL_TRACE") == "1"
    if trace:
        bass_utils.upload_artifacts = lambda tmpdir: "local://" + tmpdir
    res = bass_utils.run_bass_kernel_spmd(
        nc, in_maps, list(range(NCORES)), trace=trace)
    if trace:
        global LAST_RES
        LAST_RES = res
        print(f"HW exec time: {res.exec_time_ns} ns")

    b_out = np.asarray(b_out, dtype=np.float32)
    out = np.zeros((B, S, E), np.float32)
    for c in range(NCORES):
        out[c // GROUPS] += res.results[c]["out"].astype(np.float32)
    out += b_out[None, None, :]
    return out


# revision 25
# speedup vs baseline: 1.4062x; 1.3340x over previous
"""Multi-head attention (B=2, S=2048, E=1024, H=16) on 8 TRN2 NeuronCores.

Sharding: batch x head-group. Core c handles batch b=c//4 and head group
g=c%4 (4 heads = 256 of E). Each core computes its heads' attention output
slice and a partial fc_out product [S, E]; the host sums the 4 partials per
batch and adds b_out.

Device-side math per core:
  qpT = (Wq_g @ q[b].T + bq)      [256, S]   f32r (dims on partitions)
  kpT = (Wk_g @ k_c[b].T + bk)    [256, SKV] f32r (k compressed by mask)
  vp  = (v_c[b] @ Wv_g.T + bv)*m  [SKV, 4*65] bf16 (64 dims + ones col/head)
  per (qb, pt): S_T chunks [128kv, 512q] for both heads of the pair land in
  one 2-bank psum tile -> one Exp activation [128, 1024] -> et bf16; AV
  accumulates vp_aug.T @ E_T -> [65, 512] (row 64 = softmax denominator).
  normalize: Pool copies AV psum -> bf16; a [1,64] ones matmul broadcasts
  the denominator row; DVE reciprocal + multiply -> o_bf f16.
  fc_out accumulates BOTH pairs into one psum tile -> out [S, E] f16.

The loop is qb-major with fc(qb-1) and proj_q(qb+1) matmuls interleaved
into the attention stream so the PE never idles (p-state stays ramped).

Mask handling is exact: masked K/V rows are removed on the host (gather),
so softmax(where(mask==0, -1e20, e)) == exp(e_valid)/sum(exp(e_valid)).
"""

import os

import numpy as np

B, S, E, H = 2, 2048, 1024, 16
D = E // H           # 64
NCORES = 8
GROUPS = 4           # head groups per batch (cores per batch)
HPG = H // GROUPS    # 4 heads per core
DC = E // GROUPS     # 256 dims per core
NB = E // 128        # 8 contraction chunks over E
QB = 512             # query block
NQB = S // QB        # 4

_CACHE = {}


def _split_excess_waits(nc, max_waits=1):
    """walrus rejects instructions carrying >1 sem wait; spread extras onto
    single-wait NoOps inserted before the instruction on the same engine."""
    import concourse.mybir as mybir

    n_split = 0
    for f in nc.m.functions:
        for bb in f.blocks:
            out, changed = [], False
            for ins in bb.instructions:
                si = ins.sync_info
                if si is not None and si.on_wait is not None and len(si.on_wait) > max_waits:
                    waits = list(si.on_wait)
                    for w in waits[:-max_waits]:
                        out.append(mybir.InstNoOp(
                            name=nc.get_next_instruction_name(),
                            engine=ins.engine, ins=[], outs=[],
                            sync_info=mybir.SyncInfo(on_wait=[w], on_update=[])))
                        n_split += 1
                    ins.sync_info = mybir.SyncInfo(
                        on_wait=waits[-max_waits:], on_update=list(si.on_update))
                    changed = True
                out.append(ins)
            if changed:
                bb.instructions = out
    return n_split


def _build(skv, split_waits=True):
    import concourse.bass as bass
    import concourse.mybir as mybir
    import concourse.tile as tile

    f32 = mybir.dt.float32
    f32r = mybir.dt.float32r
    f16 = mybir.dt.float16
    bf16 = mybir.dt.bfloat16
    Alu = mybir.AluOpType
    Act = mybir.ActivationFunctionType

    nsk = skv // 128

    nc = bass.Bass()
    xqT = nc.declare_dram_parameter("xqT", [E, S], f16, isOutput=False)
    xkT = nc.declare_dram_parameter("xkT", [E, skv], f16, isOutput=False)
    xvT = nc.declare_dram_parameter("xvT", [E, skv], f16, isOutput=False)
    wqT = nc.declare_dram_parameter("wqT", [E, DC], f16, isOutput=False)
    wkT = nc.declare_dram_parameter("wkT", [E, DC], f16, isOutput=False)
    wvT = nc.declare_dram_parameter("wvT", [E, DC], f16, isOutput=False)
    woT = nc.declare_dram_parameter("woT", [DC, E], f16, isOutput=False)
    bq_d = nc.declare_dram_parameter("bq", [DC], f32, isOutput=False)
    bk_d = nc.declare_dram_parameter("bk", [DC], f32, isOutput=False)
    bv_d = nc.declare_dram_parameter("bv", [DC], f32, isOutput=False)
    vm_d = nc.declare_dram_parameter("vmask", [skv], f32, isOutput=False)
    sel_d = nc.declare_dram_parameter("sel", [2, 128], bf16, isOutput=False)
    out_d = nc.declare_dram_parameter("out", [S, E], f16, isOutput=True)

    xqT_r = xqT.rearrange("(ko p) s -> p ko s", p=128)
    xkT_r = xkT.rearrange("(ko p) s -> p ko s", p=128)
    xvT_r = xvT.rearrange("(ko p) s -> p ko s", p=128)

    with tile.TileContext(nc) as tc:
        with (
            tc.tile_pool(name="weights", bufs=4) as wpool,
            tc.tile_pool(name="consts", bufs=1) as cpool,
            tc.tile_pool(name="persist", bufs=1) as ppool,
            tc.tile_pool(name="xq", bufs=2) as xqpool,
            tc.tile_pool(name="xkv", bufs=1) as xkvpool,
            tc.tile_pool(name="sc_ps", bufs=2, space="PSUM") as scps,
            tc.tile_pool(name="av_ps", bufs=2, space="PSUM") as avps,
            tc.tile_pool(name="fc_ps", bufs=2, space="PSUM") as fps,
            tc.tile_pool(name="et", bufs=3) as etpool,
            tc.tile_pool(name="ou", bufs=3) as oupool,
            tc.tile_pool(name="rec", bufs=3) as rcpool,
            tc.tile_pool(name="outp", bufs=2) as opool,
            tc.tile_pool(name="small", bufs=2) as smpool,
        ):
            # ---- constants / weights. All big inputs go on the sync queue
            # in priority order (its FIFO then orders the transfers on the
            # bandwidth-saturated DMA engines); consts on gpsimd; scalar
            # queue stays pure-exp. k first: kpT gates attention.
            wk_t = wpool.tile([128, NB, DC], f16, tag="w", name="wk_t")
            wq_t = wpool.tile([128, NB, DC], f16, tag="w", name="wq_t")
            wv_t = wpool.tile([128, NB, DC], f16, tag="w", name="wv_t")
            wo_t = wpool.tile([128, DC // 128, E], f16, tag="w", name="wo_t")
            nc.sync.dma_start(wk_t[:], wkT.rearrange("(ko p) m -> p ko m", p=128))

            xk = xkvpool.tile([128, NB, skv], f16, tag="xk", name="xk")
            xv = xkvpool.tile([128, NB, skv], f16, tag="xv", name="xv")
            KCH = 384 if skv % 384 == 0 else 128
            nkch = skv // KCH
            for kb in range(nkch):
                nc.sync.dma_start(xk[:, :, kb * KCH:(kb + 1) * KCH],
                                  xkT_r[:, :, kb * KCH:(kb + 1) * KCH])

            xq = [xqpool.tile([128, NB, QB], f16, tag="xq", name=f"xq{nb}")
                  for nb in range(2)] + [None, None]
            nc.sync.dma_start(wq_t[:], wqT.rearrange("(ko p) m -> p ko m", p=128))
            nc.sync.dma_start(xq[0][:], xqT_r[:, :, 0:QB])
            nc.sync.dma_start(wv_t[:], wvT.rearrange("(ko p) m -> p ko m", p=128))
            for kb in range(nkch):
                nc.sync.dma_start(xv[:, :, kb * KCH:(kb + 1) * KCH],
                                  xvT_r[:, :, kb * KCH:(kb + 1) * KCH])
            nc.sync.dma_start(xq[1][:], xqT_r[:, :, QB:2 * QB])
            # bulk xq blocks 2-3: one fat-row DMA per contraction chunk
            xq23 = xqpool.tile([128, NB, S - 2 * QB], f16, tag="xqb", name="xq23")
            for ko in range(NB):
                nc.sync.dma_start(xq23[:, ko, :], xqT_r[:, ko, 2 * QB:])
            for nb in range(2, NQB):
                xq[nb] = xq23[:, :, (nb - 2) * QB:(nb - 1) * QB]
            nc.sync.dma_start(wo_t[:], woT.rearrange("(ko p) n -> p ko n", p=128))

            bq_t = cpool.tile([128, 2], f32, tag="bq")
            bk_t = cpool.tile([128, 2], f32, tag="bk")
            bv_t = cpool.tile([128, DC], f32, tag="bv")
            vm_t = cpool.tile([128, nsk], f32, tag="vm")
            nc.gpsimd.dma_start(bk_t[:], bk_d.rearrange("(c p) -> p c", p=128))
            nc.gpsimd.dma_start(bq_t[:], bq_d.rearrange("(c p) -> p c", p=128))
            nc.gpsimd.dma_start(bv_t[:], bv_d[None, :].to_broadcast((128, DC)))
            nc.gpsimd.dma_start(vm_t[:], vm_d.rearrange("(s p) -> p s", p=128))
            nc.gpsimd.dma_start(wv_t[:], wvT.rearrange("(ko p) m -> p ko m", p=128))

            # selection matrix: sel[k, p] = 1 iff p // 64 == k; broadcasts
            # rr row j to output partitions 64j..64j+63 in one matmul
            sel_t = cpool.tile([2, 128], bf16, tag="sel")
            nc.gpsimd.dma_start(sel_t[:], sel_d[:])

            qpT = ppool.tile([128, 2, S], f32r, tag="qpT")
            kpT = ppool.tile([128, 2, skv], f32r, tag="kpT")
            vp = ppool.tile([128, nsk, HPG, D + 1], bf16, tag="vp")
            o_bf = ppool.tile([128, 2, S], f16, tag="o_bf")

            # ---- projection emitters ----
            def proj_k(kb):
                off = kb * KCH
                for mc in range(2):
                    ps = fps.tile([128, 512], f32, tag="fc", name="kp_ps")[:, :KCH]
                    for kc in range(NB):
                        nc.tensor.matmul(
                            ps[:], wk_t[:, kc, mc * 128:(mc + 1) * 128],
                            xk[:, kc, off:off + KCH],
                            start=(kc == 0), stop=(kc == NB - 1))
                    nc.vector.tensor_tensor(
                        out=kpT[:, mc, off:off + KCH], in0=ps[:],
                        in1=bk_t[:, mc:mc + 1].to_broadcast((128, KCH)), op=Alu.add)

            def proj_q_mms(nb, mc, ps):
                """8 accumulating matmuls for qpT block nb, half mc."""
                for kc in range(NB):
                    yield lambda kc=kc: nc.tensor.matmul(
                        ps[:], wq_t[:, kc, mc * 128:(mc + 1) * 128],
                        xq[nb][:, kc, :], start=(kc == 0), stop=(kc == NB - 1))

            def proj_q_bias(nb, mc, ps):
                nc.vector.tensor_tensor(
                    out=qpT[:, mc, nb * QB:(nb + 1) * QB], in0=ps[:],
                    in1=bq_t[:, mc:mc + 1].to_broadcast((128, QB)), op=Alu.add)

            def proj_q(nb):
                for mc in range(2):
                    ps = fps.tile([128, 512], f32, tag="fc", name="qp_ps")
                    for mm in proj_q_mms(nb, mc, ps):
                        mm()
                    proj_q_bias(nb, mc, ps)

            def proj_v(sc):
                ps = fps.tile([128, 512], f32, tag="fc", name="vp_ps")[:, :DC]
                for kc in range(NB):
                    nc.tensor.matmul(
                        ps[:], xv[:, kc, sc * 128:(sc + 1) * 128], wv_t[:, kc, :],
                        start=(kc == 0), stop=(kc == NB - 1))
                t1 = smpool.tile([128, DC], f32, tag="vtmp")
                nc.vector.tensor_tensor(out=t1[:], in0=ps[:], in1=bv_t[:], op=Alu.add)
                nc.gpsimd.tensor_tensor(
                    out=vp[:, sc, :, 0:D],
                    in0=t1.rearrange("p (h w) -> p h w", w=D),
                    in1=vm_t[:, sc:sc + 1, None].to_broadcast((128, HPG, D)),
                    op=Alu.mult)
                nc.gpsimd.tensor_copy(
                    out=vp[:, sc, :, D:D + 1],
                    in_=vm_t[:, sc:sc + 1, None].to_broadcast((128, HPG, 1)))

            # ---- lead-in: kpT, qpT block 0 (vp is projected just-in-time
            # inside qb0/pt0's chunk loop as xv chunks stream in) ----
            for kb in range(nkch):
                proj_k(kb)
            proj_q(0)

            # ---- fc_out emitter (query row sqc: 128 queries, eb: 512 cols) --
            def fc_mms(sqc, eb, ps):
                for pt in range(2):
                    yield lambda pt=pt: nc.tensor.matmul(
                        ps[:], o_bf[:, pt, sqc * 128:(sqc + 1) * 128],
                        wo_t[:, pt, eb * 512:(eb + 1) * 512],
                        start=(pt == 0), stop=(pt == 1))

            def fc_tail(sqc, eb, ps, ob):
                nc.vector.tensor_copy(out=ob[:, eb * 512:(eb + 1) * 512], in_=ps[:])
                if eb == 1:
                    nc.sync.dma_start(
                        out_d[sqc * 128:(sqc + 1) * 128, :], ob[:])

            def fc_block(qb):
                """All fc work for query block qb as a list of thunks
                (each one PE matmul or a tail), to interleave."""
                thunks = []
                for sq in range(QB // 128):
                    sqc = qb * (QB // 128) + sq
                    ob = opool.tile([128, E], f16, tag="ob", name="ob")
                    for eb in range(2):
                        ps = fps.tile([128, 512], f32, tag="fc", name="fc_ps")
                        for mm in fc_mms(sqc, eb, ps):
                            thunks.append(mm)
                        thunks.append(
                            lambda sqc=sqc, eb=eb, ps=ps, ob=ob: fc_tail(sqc, eb, ps, ob))
                return thunks

            # ---- attention ----
            for qb in range(NQB):
                q0 = qb * QB
                # interleavable PE work: fc of previous block + proj of next q
                extra = fc_block(qb - 1) if qb > 0 else []
                if qb + 1 < NQB:
                    for mc in range(2):
                        ps = fps.tile([128, 512], f32, tag="fc", name="qp_ps")
                        for mm in proj_q_mms(qb + 1, mc, ps):
                            extra.append(mm)
                        extra.append(
                            lambda nb=qb + 1, mc=mc, ps=ps: proj_q_bias(nb, mc, ps))
                ei = 0

                def drain(n):
                    nonlocal ei
                    for _ in range(n):
                        if ei < len(extra):
                            extra[ei]()
                            ei += 1

                for pt in range(2):
                    ps_av = [avps.tile([D + 1, QB], f32, tag="av", name=f"av{j}")
                             for j in range(2)]
                    ets = []
                    for skc in range(nsk):
                        psx = scps.tile([128, 2, QB], f32, tag="sc", name="psx")
                        for j in range(2):
                            nc.tensor.matmul(
                                psx[:, j, :],
                                kpT[64 * j:64 * j + 64, pt, skc * 128:(skc + 1) * 128],
                                qpT[64 * j:64 * j + 64, pt, q0:q0 + QB],
                                start=True, stop=True, tile_position=(64 * j, 0))
                        if qb == 0 and pt == 0:
                            proj_v(skc)  # just-in-time: AV(skc) runs next step
                        et = etpool.tile([128, 2, QB], bf16, tag="et", name="et")
                        nc.scalar.activation(et[:], psx[:], Act.Exp)
                        ets.append(et)
                        # AV for previous chunk (keeps one step of slack
                        # between PE and the exp on the scalar engine)
                        if skc > 0:
                            for j in range(2):
                                nc.tensor.matmul(
                                    ps_av[j][:], vp[:, skc - 1, 2 * pt + j, :],
                                    ets[skc - 1][:, j, :],
                                    start=(skc - 1 == 0), stop=False)
                        if not (qb == 0 and pt == 0):
                            drain(2)
                    for j in range(2):
                        nc.tensor.matmul(
                            ps_av[j][:], vp[:, nsk - 1, 2 * pt + j, :],
                            ets[nsk - 1][:, j, :],
                            start=False, stop=True)
                    # normalize: Act drains AV psum to bf16; denominator rows
                    # DMA-reshaped to [128, 8] for a cheap reciprocal, then
                    # broadcast back via a [1,64] ones matmul.
                    ous = []
                    scol = rcpool.tile([128, 8], bf16, tag="scol", name="scol")
                    for j in range(2):
                        ou = oupool.tile([D + 1, QB], bf16, tag="ou", name="ou")
                        nc.vector.tensor_copy(out=ou[:], in_=ps_av[j][:])
                        ous.append(ou)
                        # denom row q=p*4+c -> scol[p, j*4+c]
                        nc.gpsimd.dma_start(
                            scol[:, j * 4:(j + 1) * 4],
                            ou[64:65, :].rearrange("o (p c) -> o p c", c=4))
                    with nc.allow_low_precision(reason="softmax denom recip"):
                        nc.vector.reciprocal(out=scol[:], in_=scol[:])
                    rr = rcpool.tile([2, QB], bf16, tag="rr", name="rr")
                    # scol[p, j*4+c] -> rr[j, p*4+c], iterating (p, c)
                    for j in range(2):
                        nc.gpsimd.dma_start(
                            rr[j:j + 1, :].rearrange("o (p c) -> o p c", c=4),
                            scol[:, j * 4:(j + 1) * 4])
                    bc = fps.tile([128, 512], f32, tag="fc", name="bc")
                    nc.tensor.matmul(bc[:], sel_t[:], rr[:],
                                     start=True, stop=True)
                    for j in range(2):
                        nc.vector.tensor_tensor(
                            out=o_bf[64 * j:64 * j + 64, pt, q0:q0 + QB],
                            in0=ous[j][0:D, :],
                            in1=bc[64 * j:64 * j + 64, :], op=Alu.mult)
                    drain(4)
                # any leftover interleaved work
                drain(len(extra))
            # final fc block
            for th in fc_block(NQB - 1):
                th()

    if split_waits:
        _split_excess_waits(nc)
    return nc


def _prep_inputs(q, k, v, mask, W_qkv, b_qkv, W_out, b_out):
    """Host-side shard/layout prep. Returns (skv, in_maps)."""
    q = np.asarray(q, dtype=np.float32)
    k = np.asarray(k, dtype=np.float32)
    v = np.asarray(v, dtype=np.float32)
    mask = np.asarray(mask)
    W_qkv = np.asarray(W_qkv, dtype=np.float32)
    b_qkv = np.asarray(b_qkv, dtype=np.float32)
    W_out = np.asarray(W_out, dtype=np.float32)

    valid = [np.nonzero(mask[b, 0, 0] != 0)[0] for b in range(B)]
    cnts = [len(vi) for vi in valid]
    skv = max(128, max((c + 127) // 128 * 128 for c in cnts))

    qT, kTc, vTc, vms = [], [], [], []
    for b in range(B):
        qT.append(np.ascontiguousarray(q[b].T).astype(np.float16))
        kt = np.zeros((E, skv), np.float16)
        vt = np.zeros((E, skv), np.float16)
        kt[:, :cnts[b]] = k[b][valid[b]].T
        vt[:, :cnts[b]] = v[b][valid[b]].T
        kTc.append(kt)
        vTc.append(vt)
        vm = np.zeros((skv,), np.float32)
        vm[:cnts[b]] = 1.0
        vms.append(vm)

    import ml_dtypes
    sel = np.zeros((2, 128), dtype=ml_dtypes.bfloat16)
    sel[0, 0:64] = 1
    sel[1, 64:128] = 1

    in_maps = []
    for c in range(NCORES):
        b, g = divmod(c, GROUPS)
        sl = slice(g * DC, (g + 1) * DC)
        in_maps.append({
            "xqT": qT[b], "xkT": kTc[b], "xvT": vTc[b],
            "wqT": np.ascontiguousarray(W_qkv[sl, :].T).astype(np.float16),
            "wkT": np.ascontiguousarray(W_qkv[E:][sl, :].T).astype(np.float16),
            "wvT": np.ascontiguousarray(W_qkv[2 * E:][sl, :].T).astype(np.float16),
            "woT": np.ascontiguousarray(W_out[:, sl].T).astype(np.float16),
            "bq": np.ascontiguousarray(b_qkv[sl]),
            "bk": np.ascontiguousarray(b_qkv[E:][sl]),
            "bv": np.ascontiguousarray(b_qkv[2 * E:][sl]),
            "vmask": vms[b],
            "sel": sel,
        })
    return skv, in_maps


def kernel(q, k, v, mask, W_qkv, b_qkv, W_out, b_out):
    from concourse import bass_utils

    skv, in_maps = _prep_inputs(q, k, v, mask, W_qkv, b_qkv, W_out, b_out)
    if skv not in _CACHE:
        _CACHE[skv] = _build(skv)
    nc = _CACHE[skv]

    trace = os.environ.get("KERNEL_TRACE") == "1"
    if trace:
        bass_utils.upload_artifacts = lambda tmpdir: "local://" + tmpdir
    res = bass_utils.run_bass_kernel_spmd(
        nc, in_maps, list(range(NCORES)), trace=trace)
    if trace:
        global LAST_RES
        LAST_RES = res
        print(f"HW exec time: {res.exec_time_ns} ns")

    b_out = np.asarray(b_out, dtype=np.float32)
    out = np.zeros((B, S, E), np.float32)
    for c in range(NCORES):
        out[c // GROUPS] += res.results[c]["out"].astype(np.float32)
    out += b_out[None, None, :]
    return out
